# revision 6
# baseline (speedup 1.0000x reference)
"""GNN message-passing kernel for Trainium2 (8 NeuronCores, data-parallel over batch).

out[b, v] = x[b, v] @ Wx + mean_k(padded[b, neighbor[v, k]]) @ Wn + bias

Strategy (per core, 2 batch elements):
  - Precompute y  = x @ (Wn/16) for both local batches, packed into an HBM
    table with 512-byte rows [y_b0[v] | y_b1[v]] (f32).  One dma_gather row
    then serves BOTH batch elements (neighbor table is batch-independent).
  - Precompute y2 = x @ Wx + bias, kept in SBUF in the same packed layout.
  - Chunked dma_gather (k-major index order) + in-place DVE binary-tree adds
    reduce the K=16 neighbor rows; add y2; DMA out.
  - x is transposed on the TensorEngine (PE) to feed the matmuls.

Index layout prep (transpose/remap of the int32 neighbor table into the
int16 [16 x N] wrapped layout dma_gather consumes) happens on host; all
data movement/compute happens on device.
"""

import numpy as np

try:
    import concourse.bass as bass
except ImportError:  # grading env may not have it on sys.path
    import sys

    sys.path.insert(0, "/opt/trn_rl_repo")
    import concourse.bass as bass

from contextlib import ExitStack

import concourse.tile as tile
from concourse import bacc, mybir
from concourse.bass_utils import run_bass_kernel_spmd
from concourse.masks import make_identity
from concourse.tile_rust import add_dep_helper

B, V, F, K, COUT = 16, 20000, 64, 16, 64
NCORES = 8
BLOC = B // NCORES  # 2 batch elements per core
VT = (V + 127) // 128  # 157 stripes of 128 vertices
VPAD = VT * 128  # 20096
ZSLOT = V  # table row holding zeros (for neighbor==0 padding)
CH = 384  # vertices per full chunk == 3 stripes
TAIL0 = VPAD - 128  # 19968
NFULL = TAIL0 // CH  # 52 full chunks, then a 128-vertex tail
# chunk list: (v0, n_vertices). Tail covers vertices 19968..20095 (padded).
CHUNKS = [(c * CH, CH) for c in range(NFULL)] + [(TAIL0, 128)]

_DT = mybir.dt
_CACHE = {}


def _build_program():
    nc = bacc.Bacc("TRN2", target_bir_lowering=False, debug=False, num_devices=NCORES)
    x_ap = nc.dram_tensor("x", [BLOC, V, F], _DT.float32, kind="ExternalInput").ap()
    wx_ap = nc.dram_tensor("wx", [F, COUT], _DT.float32, kind="ExternalInput").ap()
    wn_ap = nc.dram_tensor("wn", [F, COUT], _DT.float32, kind="ExternalInput").ap()
    b_ap = nc.dram_tensor("bias", [1, COUT], _DT.float32, kind="ExternalInput").ap()
    nb_ap = nc.dram_tensor("nbidx", [128, VPAD], _DT.int16, kind="ExternalInput").ap()
    out_ap = nc.dram_tensor(
        "out", [BLOC, V, COUT], _DT.float32, kind="ExternalOutput"
    ).ap()
    ytab_ap = nc.dram_tensor("ytab", [VPAD, 2 * COUT], _DT.float32).ap()

    with tile.TileContext(nc) as tc, ExitStack() as ctx:
        const = ctx.enter_context(tc.tile_pool(name="const", bufs=1))
        big = ctx.enter_context(tc.tile_pool(name="big", bufs=1))
        xpool = ctx.enter_context(tc.tile_pool(name="xnat", bufs=4))
        xtpool = ctx.enter_context(tc.tile_pool(name="xt", bufs=4))
        ystg = ctx.enter_context(tc.tile_pool(name="ystg", bufs=3))
        gpool = ctx.enter_context(tc.tile_pool(name="gather", bufs=2))
        opool = ctx.enter_context(tc.tile_pool(name="outstg", bufs=3))
        tpsum = ctx.enter_context(tc.tile_pool(name="tpsum", bufs=2, space="PSUM"))
        mpsum = ctx.enter_context(tc.tile_pool(name="mpsum", bufs=2, space="PSUM"))

        # ---- constants ----
        ident = const.tile([128, 128], _DT.float32)
        make_identity(nc, ident[:])
        wx_sb = const.tile([F, COUT], _DT.float32)
        nc.sync.dma_start(wx_sb[:], wx_ap[:])
        wn_sb = const.tile([F, COUT], _DT.float32)
        nc.sync.dma_start(wn_sb[:], wn_ap[:])
        wns_sb = const.tile([F, COUT], _DT.float32)
        nc.scalar.mul(wns_sb[:], wn_sb[:], 1.0 / K)  # fold the mean's 1/K into Wn
        bias_sb = const.tile([1, COUT], _DT.float32)
        nc.sync.dma_start(bias_sb[:], b_ap[:])
        ones_sb = const.tile([1, 128], _DT.float32)
        nc.gpsimd.memset(ones_sb[:], 1.0)

        nbidx_sb = big.tile([128, VPAD], _DT.int16)
        nc.sync.dma_start(nbidx_sb[:], nb_ap[:])

        # y2 = x@Wx + bias, packed [128, stripe, (b0 64 | b1 64)]
        y2_sb = big.tile([128, VT * 2 * COUT], _DT.float32)

        # ---- phase B: build xT, y table (HBM), y2 (SBUF) ----
        table_writes = []
        for t in range(VT):
            rows = 128 if t < VT - 1 else V - 128 * (VT - 1)  # last stripe: 32
            ystage = ystg.tile([128, 2 * COUT], _DT.float32)
            for b in range(BLOC):
                xnat = xpool.tile([128, F], _DT.float32)
                if rows < 128:
                    nc.gpsimd.memset(xnat[:], 0.0)
                nc.sync.dma_start(
                    xnat[:rows, :], x_ap[b, t * 128 : t * 128 + rows, :]
                )
                pt = tpsum.tile([F, 128], _DT.float32)
                nc.tensor.transpose(pt[:], xnat[:], ident[:])
                xt = xtpool.tile([F, 128], _DT.float32)
                nc.scalar.copy(xt[:], pt[:])
                yp = mpsum.tile([128, COUT], _DT.float32)
                nc.tensor.matmul(yp[:], lhsT=xt[:], rhs=wns_sb[:], start=True, stop=True)
                y2p = mpsum.tile([128, COUT], _DT.float32)
                nc.tensor.matmul(y2p[:], lhsT=xt[:], rhs=wx_sb[:], start=True, stop=False)
                nc.tensor.matmul(y2p[:], lhsT=ones_sb[:], rhs=bias_sb[:], start=False, stop=True)
                nc.scalar.copy(ystage[:, b * COUT : (b + 1) * COUT], yp[:])
                nc.scalar.copy(
                    y2_sb[:, t * 2 * COUT + b * COUT : t * 2 * COUT + (b + 1) * COUT],
                    y2p[:],
                )
            wi = nc.sync.dma_start(ytab_ap[t * 128 : (t + 1) * 128, :], ystage[:])
            table_writes.append(wi)

        # ---- phase C: gather + reduce + emit ----
        for v0, cn in CHUNKS:
            nidx = cn * K
            nblk = nidx // 128  # 48 (full) or 16 (tail)
            cb = cn // 128  # column blocks of 128 vertices: 3 or 1
            g = gpool.tile([128, 48 * 128], _DT.float32, tag="gather")
            gi = nc.gpsimd.dma_gather(
                g[:, : nblk * 128].rearrange("p (a b) -> p a b", b=2 * COUT),
                ytab_ap[:],
                nbidx_sb[:, v0 : v0 + cn],
                nidx,
                nidx,
                2 * COUT,
                single_packet=False,
            )
            for wi in table_writes:
                add_dep_helper(
                    gi.ins if hasattr(gi, "ins") else gi,
                    wi.ins if hasattr(wi, "ins") else wi,
                    reason="ytab written before gather",
                )
            # k-major block layout: block index = k*cb + j. Binary tree over k.
            half = K // 2
            while half >= 1:
                w = half * cb * 128
                nc.vector.tensor_tensor(
                    out=g[:, :w], in0=g[:, :w], in1=g[:, w : 2 * w],
                    op=mybir.AluOpType.add,
                )
                half //= 2
            osb = opool.tile([128, 3 * 128], _DT.float32, tag="outstg")
            nc.vector.tensor_tensor(
                out=osb[:, : cb * 128],
                in0=g[:, : cb * 128],
                in1=y2_sb[:, v0 * 2 * COUT // 128 : (v0 + cn) * 2 * COUT // 128],
                op=mybir.AluOpType.add,
            )
            emit_rows = min(V - v0, cn)  # tail emits only 32 real rows
            for b in range(BLOC):
                if emit_rows == cn:
                    src = osb[:, : cb * 128].rearrange("p (j c) -> p j c", c=2 * COUT)[
                        :, :, b * COUT : (b + 1) * COUT
                    ]
                    dst = out_ap[b, v0 : v0 + cn, :].rearrange(
                        "(j p) f -> p j f", p=128
                    )
                    nc.sync.dma_start(dst, src)
                else:
                    nc.sync.dma_start(
                        out_ap[b, v0 : v0 + emit_rows, :],
                        osb[:emit_rows, b * COUT : (b + 1) * COUT],
                    )

    nc.compile()
    return nc


def _prep_idx(neighbor: np.ndarray) -> np.ndarray:
    """Remap neighbor indices into table slots and lay them out in the
    [16 partitions x VPAD] wrapped order dma_gather consumes (position
    i = k*C + vlocal within each chunk -> partition i%16, column i//16),
    replicated to 128 partitions."""
    idx = np.where(neighbor == 0, ZSLOT, neighbor - 1).astype(np.int32)  # [V, K]
    idxp = np.full((VPAD, K), ZSLOT, np.int32)
    idxp[:V] = idx
    out = np.empty((16, VPAD), np.int32)
    col = 0
    for v0, cn in CHUNKS:
        blk = idxp[v0 : v0 + cn].reshape(cn // 16, 16, K)  # [j, p, k]
        out[:, col : col + cn] = blk.transpose(1, 2, 0).reshape(16, cn)
        col += cn
    assert col == VPAD
    return np.tile(out.astype(np.int16), (8, 1))


def _get_nc():
    if "nc" not in _CACHE:
        _CACHE["nc"] = _build_program()
    return _CACHE["nc"]


def make_in_maps(x, Wx, Wn, b, neighbor):
    x = np.ascontiguousarray(np.asarray(x, np.float32))
    Wx = np.ascontiguousarray(np.asarray(Wx, np.float32))
    Wn = np.ascontiguousarray(np.asarray(Wn, np.float32))
    b = np.ascontiguousarray(np.asarray(b, np.float32)).reshape(1, COUT)
    nbidx = _prep_idx(np.asarray(neighbor))
    return [
        {
            "x": x[c * BLOC : (c + 1) * BLOC],
            "wx": Wx,
            "wn": Wn,
            "bias": b,
            "nbidx": nbidx,
        }
        for c in range(NCORES)
    ]


def kernel(x, Wx, Wn, b, neighbor):
    nc = _get_nc()
    in_maps = make_in_maps(x, Wx, Wn, b, neighbor)
    res = run_bass_kernel_spmd(nc, in_maps, core_ids=list(range(NCORES)))
    return np.concatenate([res.results[c]["out"] for c in range(NCORES)], axis=0)
